# revision 10
# baseline (speedup 1.0000x reference)
"""Trainium2 Bass kernel for nn_Bottleneck (topk_masking).

out = sparsify(softmax(W_mask @ x + b_mask), k=512) * (W_up @ x + b_up)

Sharding: 8 cores = 4 batches x 2 E-halves (data-parallel batch, model-parallel E).
Each core computes its [2048, 2048] mask-logit map with an exact fp32 matmul,
spills biased raw logits to device DRAM, accumulates exp-sums on the Scalar
engine (accum_out), reduces 128-wide window maxima, selects the exact top-512
windows on-device (DVE max8 rounds + two masked gpsimd.topk runs), gathers the
raw/exp/up values for those windows, and ships only ~0.5 MB/core to the host.
The host combines the per-core partial softmax sums, ranks the gathered
candidates by their exact fp32 logits (matching jax.lax.top_k tie-order), and
scatters the 512 nonzeros per batch into the dense output.
"""
import numpy as np
from contextlib import ExitStack

B, C, E, T = 4, 512, 4096, 2048
K_SPARSE = 512
EH = E // 2          # per-core E rows
G = EH // 128        # 16 M-tiles
NW = T // 128        # 16 windows of 128 t per row
NSC = 128 * G * NW   # 32768 windows/core
NEG = -1.0e30
VOCAB = 50176        # gpsimd.topk minimum-ish vocab
NCAND = 8192         # 128 partitions x 64 max8-extracted window candidates

_CACHE = {}


def _build():
    import concourse.bass as bass
    from concourse import bacc
    import concourse.tile as tile
    import concourse.mybir as mybir
    from concourse.masks import make_identity

    f32, f16, u32 = mybir.dt.float32, mybir.dt.float16, mybir.dt.uint32

    nc = bacc.Bacc(None, target_bir_lowering=False, debug=False)

    x_in = nc.dram_tensor("x_in", [C, T], f32, kind="ExternalInput")
    wmT_in = nc.dram_tensor("wmT_in", [C, EH], f32, kind="ExternalInput")
    wup_in = nc.dram_tensor("wup_in", [EH, C], f32, kind="ExternalInput")
    bm_in = nc.dram_tensor("bm_in", [128, G], f32, kind="ExternalInput")
    pb_in = nc.dram_tensor("pb_in", [128, 2], u32, kind="ExternalInput")  # p*256, p*16

    s_out = nc.dram_tensor("s_out", [128, G], f32, kind="ExternalOutput")
    ids_out = nc.dram_tensor("ids_out", [128, 4], u32, kind="ExternalOutput")
    raw_out = nc.dram_tensor("raw_out", [128, 4, 128], f32, kind="ExternalOutput")
    expw_out = nc.dram_tensor("expw_out", [128, 4, 128], f32, kind="ExternalOutput")
    upw_out = nc.dram_tensor("upw_out", [128, 4, 128], f32, kind="ExternalOutput")

    raw_map = nc.dram_tensor("raw_map", [G, 128, T], f32)          # biased logits
    vals_dram = nc.dram_tensor("vals_dram", [VOCAB], f32)
    ids_dram = nc.dram_tensor("ids_dram", [VOCAB], u32)
    idx_dram = nc.dram_tensor("idx_dram", [512], u32)
    up_dram = nc.dram_tensor("up_dram", [512, T], f32)

    with tile.TileContext(nc) as tc:
        with ExitStack() as ctx:
            persist = ctx.enter_context(tc.tile_pool(name="persist", bufs=1))
            smax = persist.tile([128, G * NW], f32)
            s_acc = persist.tile([128, G], f32)
            xh = persist.tile([128, 4, T], f16)
            pb = persist.tile([128, 2], u32)
            nc.sync.dma_start(pb[:], pb_in.ap())

            # ---------------- phase 1: mask logits ----------------
            raw_writes = []
            with ExitStack() as c1:
                p1 = c1.enter_context(tc.tile_pool(name="p1", bufs=1))
                strips = c1.enter_context(tc.tile_pool(name="strips", bufs=3))
                psum = c1.enter_context(tc.tile_pool(name="psum1", bufs=2, space="PSUM"))
                xt = p1.tile([128, 4, T], f32)
                wt = p1.tile([128, 4, EH], f32)
                bm = p1.tile([128, G], f32)
                nc.sync.dma_start(bm[:], bm_in.ap())
                for k in range(4):
                    nc.sync.dma_start(xt[:, k, :], x_in.ap()[k * 128:(k + 1) * 128, :])
                    nc.sync.dma_start(wt[:, k, :], wmT_in.ap()[k * 128:(k + 1) * 128, :])
                nc.vector.tensor_copy(xh[:], xt[:])  # fp16 copy for the up matmul

                for g in range(G):
                    acc = psum.tile([128, T], f32)
                    for n in range(4):
                        for k in range(4):
                            nc.tensor.matmul(
                                acc[:, n * 512:(n + 1) * 512],
                                wt[:, k, g * 128:(g + 1) * 128],
                                xt[:, k, n * 512:(n + 1) * 512],
                                start=(k == 0),
                                stop=(k == 3),
                            )
                    raw = strips.tile([128, T], f32, tag="raw")
                    nc.scalar.activation(
                        raw[:], acc[:], mybir.ActivationFunctionType.Identity,
                        bias=bm[:, g:g + 1], scale=1.0,
                    )
                    exps = strips.tile([128, T], f32, tag="exp")
                    nc.scalar.activation(
                        exps[:], acc[:], mybir.ActivationFunctionType.Exp,
                        bias=bm[:, g:g + 1], scale=1.0, accum_out=s_acc[:, g:g + 1],
                    )
                    raw_writes.append(nc.sync.dma_start(raw_map.ap()[g], raw[:]))
                    nc.vector.tensor_reduce(
                        smax[:, g * NW:(g + 1) * NW],
                        raw[:].rearrange("p (w i) -> p w i", i=128),
                        axis=mybir.AxisListType.X,
                        op=mybir.AluOpType.max,
                    )
            nc.sync.dma_start(s_out.ap(), s_acc[:])

            # ---------------- phase 2: select top-512 windows ----------------
            with ExitStack() as c2:
                p2 = c2.enter_context(tc.tile_pool(name="p2", bufs=1))
                mxv = p2.tile([128, 64], f32)
                mxi = p2.tile([128, 64], u32)
                for r in range(8):
                    nc.vector.max(out=mxv[:, r * 8:(r + 1) * 8], in_=smax[:])
                    nc.vector.max_index(
                        out=mxi[:, r * 8:(r + 1) * 8],
                        in_max=mxv[:, r * 8:(r + 1) * 8],
                        in_values=smax[:],
                    )
                    nc.vector.match_replace(
                        out=smax[:], in_to_replace=mxv[:, r * 8:(r + 1) * 8],
                        in_values=smax[:], imm_value=NEG,
                    )
                cid = p2.tile([128, 64], u32)
                nc.vector.tensor_tensor(
                    cid[:], mxi[:], pb[:, 0:1].to_broadcast([128, 64]),
                    op=mybir.AluOpType.add,
                )
                from bass_rust import add_dep_helper

                PAD = VOCAB - NCAND  # 41984 = 16*2624
                negt = p2.tile([16, PAD // 16], f32)
                nc.vector.memset(negt[:], NEG)
                # disjoint regions: pad-fill [NCAND:], candidates [0:NCAND]
                w_pad = nc.sync.dma_start(
                    vals_dram.ap()[NCAND:].rearrange("(p f) -> p f", p=16), negt[:]
                )
                # transpose stream order (c = r*128 + p): gpsimd.topk mis-selects
                # when the input contains long monotone runs (sorted max8 rows)
                w_val = nc.sync.dma_start(
                    vals_dram.ap()[0:NCAND].rearrange("(f p) -> p f", p=128), mxv[:]
                )
                w_ids = nc.sync.dma_start(
                    ids_dram.ap()[0:NCAND].rearrange("(f p) -> p f", p=128), cid[:]
                )
                tkin = nc.alloc_sbuf_tensor("tkin", [16, VOCAB // 16], f32).ap()
                r_tk = nc.sync.dma_start(
                    tkin[:], vals_dram.ap().rearrange("(p f) -> p f", p=16)
                )
                add_dep_helper(r_tk.ins, w_pad.ins, reason="vals RAW pad")
                add_dep_helper(r_tk.ins, w_val.ins, reason="vals RAW cand")
                tko1 = nc.alloc_sbuf_tensor("tko1", [16, 32], u32).ap()
                nc.gpsimd.topk(tko1[:], tkin[:], tokens=1, vocab_size=VOCAB, k=256)
                w_i1 = nc.sync.dma_start(
                    idx_dram.ap()[0:256].rearrange("(p f) -> p f", p=16), tko1[:, 16:32]
                )
                idx1 = p2.tile([128, 2], u32)
                r_i1 = nc.sync.dma_start(
                    idx1[:], idx_dram.ap()[0:256].rearrange("(p f) -> p f", p=128)
                )
                add_dep_helper(r_i1.ins, w_i1.ins, reason="idx RAW")
                negs = p2.tile([128, 1], f32)
                nc.vector.memset(negs[:], NEG)
                scatters = []
                for j in range(2):
                    sc = nc.gpsimd.indirect_dma_start(
                        out=vals_dram.ap().rearrange("(v one) -> v one", one=1),
                        out_offset=bass.IndirectOffsetOnAxis(ap=idx1[:, j:j + 1], axis=0),
                        in_=negs[:],
                        in_offset=None,
                    )
                    add_dep_helper(sc.ins, r_tk.ins, reason="vals WAR topk1 input")
                    scatters.append(sc)
                tkin2 = nc.alloc_sbuf_tensor("tkin2", [16, VOCAB // 16], f32).ap()
                r_tk2 = nc.sync.dma_start(
                    tkin2[:], vals_dram.ap().rearrange("(p f) -> p f", p=16)
                )
                for sc in scatters:
                    add_dep_helper(r_tk2.ins, sc.ins, reason="vals RAW masked")
                tko2 = nc.alloc_sbuf_tensor("tko2", [16, 32], u32).ap()
                nc.gpsimd.topk(tko2[:], tkin2[:], tokens=1, vocab_size=VOCAB, k=256)
                w_i2 = nc.sync.dma_start(
                    idx_dram.ap()[256:512].rearrange("(p f) -> p f", p=16), tko2[:, 16:32]
                )
                idx_all = p2.tile([128, 4], u32)
                r_ia = nc.sync.dma_start(
                    idx_all[:], idx_dram.ap().rearrange("(p f) -> p f", p=128)
                )
                add_dep_helper(r_ia.ins, w_i1.ins, reason="idx RAW 1")
                add_dep_helper(r_ia.ins, w_i2.ins, reason="idx RAW 2")
                sel = persist.tile([128, 4], u32)
                for j in range(4):
                    ga = nc.gpsimd.indirect_dma_start(
                        out=sel[:, j:j + 1],
                        out_offset=None,
                        in_=ids_dram.ap().rearrange("(v one) -> v one", one=1),
                        in_offset=bass.IndirectOffsetOnAxis(ap=idx_all[:, j:j + 1], axis=0),
                    )
                    add_dep_helper(ga.ins, w_ids.ins, reason="ids RAW")
                nc.sync.dma_start(ids_out.ap(), sel[:])

            # ---------------- phase 3: gather windows + up values ----------------
            with ExitStack() as c3:
                p3 = c3.enter_context(tc.tile_pool(name="p3", bufs=1))
                psum3 = c3.enter_context(tc.tile_pool(name="psum3", bufs=2, space="PSUM"))
                shr, band, mul, add = (
                    mybir.AluOpType.logical_shift_right,
                    mybir.AluOpType.bitwise_and,
                    mybir.AluOpType.mult,
                    mybir.AluOpType.add,
                )
                tg = p3.tile([128, 4], u32)   # (id>>4)&15  = g
                tp = p3.tile([128, 4], u32)   # id>>8       = p
                tj = p3.tile([128, 4], u32)   # id&15       = j window
                r3 = p3.tile([128, 4], u32)
                i2 = p3.tile([128, 4], u32)
                er = p3.tile([128, 4], u32)
                nc.vector.tensor_scalar(tg[:], sel[:], 4, 15, op0=shr, op1=band)
                nc.vector.tensor_scalar(tp[:], sel[:], 8, None, op0=shr)
                nc.vector.tensor_scalar(tj[:], sel[:], 15, None, op0=band)
                # r3 = g*2048 + p*16 + j   (raw_map window row)
                nc.vector.tensor_scalar(r3[:], tg[:], 2048, None, op0=mul)
                nc.vector.tensor_scalar(i2[:], tp[:], 16, None, op0=mul)
                nc.vector.tensor_tensor(r3[:], r3[:], i2[:], op=add)
                nc.vector.tensor_tensor(r3[:], r3[:], tj[:], op=add)
                # e_row = g*128 + p        (W_up row)
                nc.vector.tensor_scalar(er[:], tg[:], 128, None, op0=mul)
                nc.vector.tensor_tensor(er[:], er[:], tp[:], op=add)
                # idx2 = j_col*2048 + p*16 + j  (up_dram window row; q = j_col*128+p)
                nc.vector.tensor_tensor(
                    i2[:], tj[:], pb[:, 1:2].to_broadcast([128, 4]), op=add
                )
                for j in range(4):
                    nc.vector.tensor_scalar(
                        i2[:, j:j + 1], i2[:, j:j + 1], j * 2048, None, op0=add
                    )

                # gather W_up rows for the 512 selected windows
                wg = p3.tile([128, 4, C], f32)
                for j in range(4):
                    nc.gpsimd.indirect_dma_start(
                        out=wg[:, j, :],
                        out_offset=None,
                        in_=wup_in.ap(),
                        in_offset=bass.IndirectOffsetOnAxis(ap=er[:, j:j + 1], axis=0),
                    )
                # transpose to lhsT layout [c, q] (q = j*128 + p), fp16
                ident = p3.tile([128, 128], f32)
                make_identity(nc, ident[:])
                wgt = p3.tile([128, 4, 512], f16)
                for j in range(4):
                    for kc in range(4):
                        tp_ps = psum3.tile([128, 128], f32, tag="ps3")
                        nc.tensor.transpose(
                            tp_ps[:], wg[:, j, kc * 128:(kc + 1) * 128], ident[:]
                        )
                        nc.vector.tensor_copy(wgt[:, kc, j * 128:(j + 1) * 128], tp_ps[:])
                # up rows: [512, T] = wgt.T @ xh  (fp16 inputs, fp32 accum)
                upr = p3.tile([128, 4, T], f32)
                for m in range(4):
                    upp = psum3.tile([128, T], f32, tag="ps3")
                    for n in range(4):
                        for k in range(4):
                            nc.tensor.matmul(
                                upp[:, n * 512:(n + 1) * 512],
                                wgt[:, k, m * 128:(m + 1) * 128],
                                xh[:, k, n * 512:(n + 1) * 512],
                                start=(k == 0),
                                stop=(k == 3),
                            )
                    nc.scalar.copy(upr[:, m, :], upp[:])
                from bass_rust import add_dep_helper as _adh
                up_writes = []
                for m in range(4):
                    up_writes.append(nc.sync.dma_start(
                        up_dram.ap()[m * 128:(m + 1) * 128, :], upr[:, m, :]
                    ))
                # window gathers
                rww = p3.tile([128, 4, 128], f32)
                upw = p3.tile([128, 4, 128], f32)
                for j in range(4):
                    g_r = nc.gpsimd.indirect_dma_start(
                        out=rww[:, j, :],
                        out_offset=None,
                        in_=raw_map.ap().rearrange("g p (w i) -> (g p w) i", i=128),
                        in_offset=bass.IndirectOffsetOnAxis(ap=r3[:, j:j + 1], axis=0),
                    )
                    for w in raw_writes:
                        _adh(g_r.ins, w.ins, reason="raw_map RAW")
                    g_u = nc.gpsimd.indirect_dma_start(
                        out=upw[:, j, :],
                        out_offset=None,
                        in_=up_dram.ap().rearrange("q (w i) -> (q w) i", i=128),
                        in_offset=bass.IndirectOffsetOnAxis(ap=i2[:, j:j + 1], axis=0),
                    )
                    for w in up_writes:
                        _adh(g_u.ins, w.ins, reason="up_dram RAW")
                expw = p3.tile([128, 4, 128], f32)
                nc.scalar.activation(
                    expw[:], rww[:], mybir.ActivationFunctionType.Exp,
                    bias=0.0, scale=1.0,
                )
                nc.sync.dma_start(raw_out.ap(), rww[:])
                nc.sync.dma_start(upw_out.ap(), upw[:])
                nc.sync.dma_start(expw_out.ap(), expw[:])

    nc.compile()
    return nc


def _get_runner():
    if "runner" in _CACHE:
        return _CACHE["runner"]
    from concourse.bass_utils import run_bass_kernel_spmd

    nc = _build()

    def run(in_maps):
        return run_bass_kernel_spmd(nc, in_maps, core_ids=list(range(8))).results

    _CACHE["runner"] = run
    return run


def kernel(x, W_up, b_up, W_mask, b_mask):
    x = np.ascontiguousarray(np.asarray(x, np.float32))
    W_up = np.ascontiguousarray(np.asarray(W_up, np.float32))
    b_up = np.asarray(b_up, np.float32)
    W_mask = np.asarray(W_mask, np.float32)
    b_mask = np.asarray(b_mask, np.float32)

    pb = np.stack(
        [np.arange(128, dtype=np.uint32) * 256, np.arange(128, dtype=np.uint32) * 16],
        axis=1,
    )
    in_maps = []
    for core in range(8):
        b, h = divmod(core, 2)
        sl = slice(h * EH, (h + 1) * EH)
        in_maps.append({
            "x_in": x[b],
            "wmT_in": np.ascontiguousarray(W_mask[sl].T),
            "wup_in": W_up[sl],
            "bm_in": np.ascontiguousarray(b_mask[sl].reshape(G, 128).T),
            "pb_in": pb,
        })

    results = _get_runner()(in_maps)

    out = np.zeros((B, E, T), dtype=np.float32)
    for b in range(B):
        Z = 0.0
        for h in range(2):
            Z += np.asarray(results[2 * b + h]["s_out"], np.float64).sum()
        cand_val = []
        cand_pos = []
        cand_out = []
        for h in range(2):
            r = results[2 * b + h]
            ids = r["ids_out"].reshape(-1).astype(np.int64)        # [512]
            rww = r["raw_out"].reshape(128, 4, 128)
            expw = r["expw_out"].reshape(128, 4, 128).astype(np.float64)
            upw = r["upw_out"].reshape(128, 4, 128)
            g = (ids >> 4) & 15
            p = ids >> 8
            j = ids & 15
            e = h * EH + g * 128 + p                               # [512]
            t0 = j * 128
            upb = upw.reshape(512, 128) + b_up[e][:, None]
            vals = (expw.reshape(512, 128) / Z) * upb              # output values
            pos = (e[:, None] * T + (t0[:, None] + np.arange(128))).reshape(-1)
            cand_val.append(rww.reshape(-1))
            cand_pos.append(pos)
            cand_out.append(vals.reshape(-1))
        cv = np.concatenate(cand_val)
        cp = np.concatenate(cand_pos)
        co = np.concatenate(cand_out)
        # rank by exact fp32 logit, ties broken by lower flat index (jax order)
        order = np.lexsort((cp, -cv))[:K_SPARSE]
        out[b].reshape(-1)[cp[order]] = co[order].astype(np.float32)
    return out


# revision 13
# speedup vs baseline: 1.2229x; 1.2229x over previous
"""Trainium2 Bass kernel for nn_Bottleneck (topk_masking).

out = sparsify(softmax(W_mask @ x + b_mask), k=512) * (W_up @ x + b_up)

Sharding: 8 cores = 4 batches x 2 E-halves (data-parallel batch, model-parallel E).
Each core computes its [2048, 2048] mask-logit map with an exact fp32 matmul,
spills biased raw logits to device DRAM, accumulates exp-sums on the Scalar
engine (accum_out), reduces 128-wide window maxima, selects the exact top-512
windows on-device (DVE max8 rounds + two masked gpsimd.topk runs), gathers the
raw/exp/up values for those windows, and ships only ~0.5 MB/core to the host.
The host combines the per-core partial softmax sums, ranks the gathered
candidates by their exact fp32 logits (matching jax.lax.top_k tie-order), and
scatters the 512 nonzeros per batch into the dense output.
"""
import numpy as np
from contextlib import ExitStack

B, C, E, T = 4, 512, 4096, 2048
K_SPARSE = 512
EH = E // 2          # per-core E rows
G = EH // 128        # 16 M-tiles
NW = T // 128        # 16 windows of 128 t per row
NSC = 128 * G * NW   # 32768 windows/core
NEG = -1.0e30
VOCAB = 50176        # gpsimd.topk minimum-ish vocab
NCAND = 8192         # 128 partitions x 64 max8-extracted window candidates

_CACHE = {}


def _build():
    import concourse.bass as bass
    from concourse import bacc
    import concourse.tile as tile
    import concourse.mybir as mybir
    from concourse.masks import make_identity

    f32, f16, u32 = mybir.dt.float32, mybir.dt.float16, mybir.dt.uint32

    nc = bacc.Bacc(None, target_bir_lowering=False, debug=False)

    x_in = nc.dram_tensor("x_in", [C, T], f32, kind="ExternalInput")
    wmT_in = nc.dram_tensor("wmT_in", [C, EH], f32, kind="ExternalInput")
    wup_in = nc.dram_tensor("wup_in", [EH, C], f32, kind="ExternalInput")
    bm_in = nc.dram_tensor("bm_in", [128, G], f32, kind="ExternalInput")
    pb_in = nc.dram_tensor("pb_in", [128, 2], u32, kind="ExternalInput")  # p*256, p*16

    s_out = nc.dram_tensor("s_out", [128, G], f32, kind="ExternalOutput")
    ids_out = nc.dram_tensor("ids_out", [128, 4], u32, kind="ExternalOutput")
    raw_out = nc.dram_tensor("raw_out", [128, 4, 128], f32, kind="ExternalOutput")
    expw_out = nc.dram_tensor("expw_out", [128, 4, 128], f32, kind="ExternalOutput")
    upw_out = nc.dram_tensor("upw_out", [128, 4, 128], f32, kind="ExternalOutput")

    raw_map = nc.dram_tensor("raw_map", [G, 128, T], f32)          # biased logits
    vals_dram = nc.dram_tensor("vals_dram", [VOCAB], f32)
    ids_dram = nc.dram_tensor("ids_dram", [VOCAB], u32)
    idx_dram = nc.dram_tensor("idx_dram", [512], u32)
    up_dram = nc.dram_tensor("up_dram", [512, T], f32)

    with tile.TileContext(nc) as tc:
        with ExitStack() as ctx:
            persist = ctx.enter_context(tc.tile_pool(name="persist", bufs=1))
            smax = persist.tile([128, G * NW], f32)
            s_acc = persist.tile([128, G], f32)
            xh = persist.tile([128, 4, T], f16)
            pb = persist.tile([128, 2], u32)
            nc.sync.dma_start(pb[:], pb_in.ap())

            # ---------------- phase 1: mask logits ----------------
            raw_writes = []
            with ExitStack() as c1:
                p1 = c1.enter_context(tc.tile_pool(name="p1", bufs=1))
                strips = c1.enter_context(tc.tile_pool(name="strips", bufs=3))
                psum = c1.enter_context(tc.tile_pool(name="psum1", bufs=2, space="PSUM"))
                xt = p1.tile([128, 4, T], f32)
                wt = p1.tile([128, 4, EH], f32)
                bm = p1.tile([128, G], f32)
                nc.sync.dma_start(bm[:], bm_in.ap())
                for k in range(4):
                    nc.sync.dma_start(xt[:, k, :], x_in.ap()[k * 128:(k + 1) * 128, :])
                    nc.sync.dma_start(wt[:, k, :], wmT_in.ap()[k * 128:(k + 1) * 128, :])
                nc.vector.tensor_copy(xh[:], xt[:])  # fp16 copy for the up matmul

                for g in range(G):
                    acc = psum.tile([128, T], f32)
                    for n in range(4):
                        for k in range(4):
                            nc.tensor.matmul(
                                acc[:, n * 512:(n + 1) * 512],
                                wt[:, k, g * 128:(g + 1) * 128],
                                xt[:, k, n * 512:(n + 1) * 512],
                                start=(k == 0),
                                stop=(k == 3),
                            )
                    raw = strips.tile([128, T], f32, tag="raw")
                    nc.scalar.activation(
                        raw[:], acc[:], mybir.ActivationFunctionType.Identity,
                        bias=bm[:, g:g + 1], scale=1.0,
                    )
                    exps = strips.tile([128, T], f32, tag="exp")
                    nc.scalar.activation(
                        exps[:], acc[:], mybir.ActivationFunctionType.Exp,
                        bias=bm[:, g:g + 1], scale=1.0, accum_out=s_acc[:, g:g + 1],
                    )
                    raw_writes.append(nc.sync.dma_start(raw_map.ap()[g], raw[:]))
                    nc.vector.tensor_reduce(
                        smax[:, g * NW:(g + 1) * NW],
                        raw[:].rearrange("p (w i) -> p w i", i=128),
                        axis=mybir.AxisListType.X,
                        op=mybir.AluOpType.max,
                    )
            nc.sync.dma_start(s_out.ap(), s_acc[:])

            # ---------------- phase 2: select top-512 windows ----------------
            with ExitStack() as c2:
                p2 = c2.enter_context(tc.tile_pool(name="p2", bufs=1))
                mxv = p2.tile([128, 64], f32)
                mxi = p2.tile([128, 64], u32)
                for r in range(8):
                    nc.vector.max(out=mxv[:, r * 8:(r + 1) * 8], in_=smax[:])
                    nc.vector.max_index(
                        out=mxi[:, r * 8:(r + 1) * 8],
                        in_max=mxv[:, r * 8:(r + 1) * 8],
                        in_values=smax[:],
                    )
                    nc.vector.match_replace(
                        out=smax[:], in_to_replace=mxv[:, r * 8:(r + 1) * 8],
                        in_values=smax[:], imm_value=NEG,
                    )
                cid = p2.tile([128, 64], u32)
                nc.vector.tensor_tensor(
                    cid[:], mxi[:], pb[:, 0:1].to_broadcast([128, 64]),
                    op=mybir.AluOpType.add,
                )
                from bass_rust import add_dep_helper

                PAD = VOCAB - NCAND  # 41984 = 16*2624
                negt = p2.tile([16, PAD // 16], f32)
                nc.vector.memset(negt[:], NEG)
                # disjoint regions: pad-fill [NCAND:], candidates [0:NCAND]
                w_pad = nc.sync.dma_start(
                    vals_dram.ap()[NCAND:].rearrange("(p f) -> p f", p=16), negt[:]
                )
                # transpose stream order (c = r*128 + p): gpsimd.topk mis-selects
                # when the input contains long monotone runs (sorted max8 rows)
                w_val = nc.sync.dma_start(
                    vals_dram.ap()[0:NCAND].rearrange("(f p) -> p f", p=128), mxv[:]
                )
                w_ids = nc.sync.dma_start(
                    ids_dram.ap()[0:NCAND].rearrange("(f p) -> p f", p=128), cid[:]
                )
                tkin = nc.alloc_sbuf_tensor("tkin", [16, VOCAB // 16], f32).ap()
                r_tk = nc.sync.dma_start(
                    tkin[:], vals_dram.ap().rearrange("(p f) -> p f", p=16)
                )
                add_dep_helper(r_tk.ins, w_pad.ins, reason="vals RAW pad")
                add_dep_helper(r_tk.ins, w_val.ins, reason="vals RAW cand")
                tko1 = nc.alloc_sbuf_tensor("tko1", [16, 32], u32).ap()
                nc.gpsimd.topk(tko1[:], tkin[:], tokens=1, vocab_size=VOCAB, k=256)
                w_i1 = nc.sync.dma_start(
                    idx_dram.ap()[0:256].rearrange("(p f) -> p f", p=16), tko1[:, 16:32]
                )
                idx1 = p2.tile([128, 2], u32)
                r_i1 = nc.sync.dma_start(
                    idx1[:], idx_dram.ap()[0:256].rearrange("(p f) -> p f", p=128)
                )
                add_dep_helper(r_i1.ins, w_i1.ins, reason="idx RAW")
                negs = p2.tile([128, 1], f32)
                nc.vector.memset(negs[:], NEG)
                scatters = []
                for j in range(2):
                    sc = nc.gpsimd.indirect_dma_start(
                        out=vals_dram.ap().rearrange("(v one) -> v one", one=1),
                        out_offset=bass.IndirectOffsetOnAxis(ap=idx1[:, j:j + 1], axis=0),
                        in_=negs[:],
                        in_offset=None,
                    )
                    add_dep_helper(sc.ins, r_tk.ins, reason="vals WAR topk1 input")
                    scatters.append(sc)
                tkin2 = nc.alloc_sbuf_tensor("tkin2", [16, VOCAB // 16], f32).ap()
                r_tk2 = nc.sync.dma_start(
                    tkin2[:], vals_dram.ap().rearrange("(p f) -> p f", p=16)
                )
                for sc in scatters:
                    add_dep_helper(r_tk2.ins, sc.ins, reason="vals RAW masked")
                tko2 = nc.alloc_sbuf_tensor("tko2", [16, 32], u32).ap()
                nc.gpsimd.topk(tko2[:], tkin2[:], tokens=1, vocab_size=VOCAB, k=256)
                w_i2 = nc.sync.dma_start(
                    idx_dram.ap()[256:512].rearrange("(p f) -> p f", p=16), tko2[:, 16:32]
                )
                idx_all = p2.tile([128, 4], u32)
                r_ia = nc.sync.dma_start(
                    idx_all[:], idx_dram.ap().rearrange("(p f) -> p f", p=128)
                )
                add_dep_helper(r_ia.ins, w_i1.ins, reason="idx RAW 1")
                add_dep_helper(r_ia.ins, w_i2.ins, reason="idx RAW 2")
                sel = persist.tile([128, 4], u32)
                for j in range(4):
                    ga = nc.gpsimd.indirect_dma_start(
                        out=sel[:, j:j + 1],
                        out_offset=None,
                        in_=ids_dram.ap().rearrange("(v one) -> v one", one=1),
                        in_offset=bass.IndirectOffsetOnAxis(ap=idx_all[:, j:j + 1], axis=0),
                    )
                    add_dep_helper(ga.ins, w_ids.ins, reason="ids RAW")
                nc.sync.dma_start(ids_out.ap(), sel[:])

            # ---------------- phase 3: gather windows + up values ----------------
            with ExitStack() as c3:
                p3 = c3.enter_context(tc.tile_pool(name="p3", bufs=1))
                psum3 = c3.enter_context(tc.tile_pool(name="psum3", bufs=2, space="PSUM"))
                shr, band, mul, add = (
                    mybir.AluOpType.logical_shift_right,
                    mybir.AluOpType.bitwise_and,
                    mybir.AluOpType.mult,
                    mybir.AluOpType.add,
                )
                tg = p3.tile([128, 4], u32)   # (id>>4)&15  = g
                tp = p3.tile([128, 4], u32)   # id>>8       = p
                tj = p3.tile([128, 4], u32)   # id&15       = j window
                r3 = p3.tile([128, 4], u32)
                i2 = p3.tile([128, 4], u32)
                er = p3.tile([128, 4], u32)
                nc.vector.tensor_scalar(tg[:], sel[:], 4, 15, op0=shr, op1=band)
                nc.vector.tensor_scalar(tp[:], sel[:], 8, None, op0=shr)
                nc.vector.tensor_scalar(tj[:], sel[:], 15, None, op0=band)
                # r3 = g*2048 + p*16 + j   (raw_map window row)
                nc.vector.tensor_scalar(r3[:], tg[:], 2048, None, op0=mul)
                nc.vector.tensor_scalar(i2[:], tp[:], 16, None, op0=mul)
                nc.vector.tensor_tensor(r3[:], r3[:], i2[:], op=add)
                nc.vector.tensor_tensor(r3[:], r3[:], tj[:], op=add)
                # e_row = g*128 + p        (W_up row)
                nc.vector.tensor_scalar(er[:], tg[:], 128, None, op0=mul)
                nc.vector.tensor_tensor(er[:], er[:], tp[:], op=add)
                # idx2 = j_col*2048 + p*16 + j  (up_dram window row; q = j_col*128+p)
                nc.vector.tensor_tensor(
                    i2[:], tj[:], pb[:, 1:2].to_broadcast([128, 4]), op=add
                )
                for j in range(4):
                    nc.vector.tensor_scalar(
                        i2[:, j:j + 1], i2[:, j:j + 1], j * 2048, None, op0=add
                    )

                # gather W_up rows for the 512 selected windows
                wg = p3.tile([128, 4, C], f32)
                for j in range(4):
                    nc.gpsimd.indirect_dma_start(
                        out=wg[:, j, :],
                        out_offset=None,
                        in_=wup_in.ap(),
                        in_offset=bass.IndirectOffsetOnAxis(ap=er[:, j:j + 1], axis=0),
                    )
                # transpose to lhsT layout [c, q] (q = j*128 + p), fp16
                ident = p3.tile([128, 128], f32)
                make_identity(nc, ident[:])
                wgt = p3.tile([128, 4, 512], f16)
                for j in range(4):
                    for kc in range(4):
                        tp_ps = psum3.tile([128, 128], f32, tag="ps3")
                        nc.tensor.transpose(
                            tp_ps[:], wg[:, j, kc * 128:(kc + 1) * 128], ident[:]
                        )
                        nc.vector.tensor_copy(wgt[:, kc, j * 128:(j + 1) * 128], tp_ps[:])
                # up rows: [512, T] = wgt.T @ xh  (fp16 inputs, fp32 accum)
                upr = p3.tile([128, 4, T], f32)
                for m in range(4):
                    upp = psum3.tile([128, T], f32, tag="ps3")
                    for n in range(4):
                        for k in range(4):
                            nc.tensor.matmul(
                                upp[:, n * 512:(n + 1) * 512],
                                wgt[:, k, m * 128:(m + 1) * 128],
                                xh[:, k, n * 512:(n + 1) * 512],
                                start=(k == 0),
                                stop=(k == 3),
                            )
                    nc.scalar.copy(upr[:, m, :], upp[:])
                from bass_rust import add_dep_helper as _adh
                up_writes = []
                for m in range(4):
                    up_writes.append(nc.sync.dma_start(
                        up_dram.ap()[m * 128:(m + 1) * 128, :], upr[:, m, :]
                    ))
                # window gathers
                rww = p3.tile([128, 4, 128], f32)
                upw = p3.tile([128, 4, 128], f32)
                for j in range(4):
                    g_r = nc.gpsimd.indirect_dma_start(
                        out=rww[:, j, :],
                        out_offset=None,
                        in_=raw_map.ap().rearrange("g p (w i) -> (g p w) i", i=128),
                        in_offset=bass.IndirectOffsetOnAxis(ap=r3[:, j:j + 1], axis=0),
                    )
                    for w in raw_writes:
                        _adh(g_r.ins, w.ins, reason="raw_map RAW")
                    g_u = nc.gpsimd.indirect_dma_start(
                        out=upw[:, j, :],
                        out_offset=None,
                        in_=up_dram.ap().rearrange("q (w i) -> (q w) i", i=128),
                        in_offset=bass.IndirectOffsetOnAxis(ap=i2[:, j:j + 1], axis=0),
                    )
                    for w in up_writes:
                        _adh(g_u.ins, w.ins, reason="up_dram RAW")
                expw = p3.tile([128, 4, 128], f32)
                nc.scalar.activation(
                    expw[:], rww[:], mybir.ActivationFunctionType.Exp,
                    bias=0.0, scale=1.0,
                )
                nc.sync.dma_start(raw_out.ap(), rww[:])
                nc.sync.dma_start(upw_out.ap(), upw[:])
                nc.sync.dma_start(expw_out.ap(), expw[:])

    nc.compile()
    return nc


def _get_runner():
    """Build the Bass program once and wrap it in a cached jax.jit/shard_map
    callable (mirrors concourse.bass2jax.run_bass_via_pjrt, but reusable so
    repeat calls skip retracing/lowering)."""
    if "runner" in _CACHE:
        return _CACHE["runner"]
    import jax
    import jax.numpy as jnp
    from jax.sharding import Mesh, PartitionSpec
    from jax.experimental.shard_map import shard_map
    import concourse.mybir as mybir
    from concourse import bass2jax
    from concourse.bass2jax import _bass_exec_p, install_neuronx_cc_hook

    install_neuronx_cc_hook()
    nc = _build()
    n_cores = 8
    partition_name = nc.partition_id_tensor.name if nc.partition_id_tensor else None

    in_names, out_names, out_avals, zero_shapes = [], [], [], []
    for alloc in nc.m.functions[0].allocations:
        if not isinstance(alloc, mybir.MemoryLocationSet):
            continue
        name = alloc.memorylocations[0].name
        if alloc.kind == "ExternalInput":
            if name != partition_name:
                in_names.append(name)
        elif alloc.kind == "ExternalOutput":
            shape = tuple(alloc.tensor_shape)
            dtype = mybir.dt.np(alloc.dtype)
            out_names.append(name)
            out_avals.append(jax.core.ShapedArray(shape, dtype))
            zero_shapes.append((shape, dtype))
    n_params = len(in_names)
    n_outs = len(out_avals)
    all_names = in_names + out_names
    if partition_name is not None:
        all_names = all_names + [partition_name]
    donate = tuple(range(n_params, n_params + n_outs))

    def _body(*args):
        operands = list(args)
        if partition_name is not None:
            operands.append(bass2jax.partition_id_tensor())
        outs = _bass_exec_p.bind(
            *operands,
            out_avals=tuple(out_avals),
            in_names=tuple(all_names),
            out_names=tuple(out_names),
            lowering_input_output_aliases=(),
            sim_require_finite=True,
            sim_require_nnan=True,
            nc=nc,
        )
        return tuple(outs)

    devices = jax.devices()[:n_cores]
    mesh = Mesh(np.asarray(devices), ("core",))
    sharded = jax.jit(
        shard_map(
            _body, mesh=mesh,
            in_specs=(PartitionSpec("core"),) * (n_params + n_outs),
            out_specs=(PartitionSpec("core"),) * n_outs,
            check_rep=False,
        ),
        donate_argnums=donate,
        keep_unused=True,
    )

    def run(in_maps):
        concat_in = [
            np.concatenate([np.asarray(m[name]) for m in in_maps], axis=0)
            for name in in_names
        ]
        concat_zeros = [
            np.zeros((n_cores * s[0], *s[1:]), d) for (s, d) in zero_shapes
        ]
        out_arrs = sharded(*concat_in, *concat_zeros)
        return [
            {
                name: np.asarray(out_arrs[i]).reshape(n_cores, *out_avals[i].shape)[c]
                for i, name in enumerate(out_names)
            }
            for c in range(n_cores)
        ]

    _CACHE["runner"] = run
    return run


def kernel(x, W_up, b_up, W_mask, b_mask):
    x = np.ascontiguousarray(np.asarray(x, np.float32))
    W_up = np.ascontiguousarray(np.asarray(W_up, np.float32))
    b_up = np.asarray(b_up, np.float32)
    W_mask = np.asarray(W_mask, np.float32)
    b_mask = np.asarray(b_mask, np.float32)

    pb = np.stack(
        [np.arange(128, dtype=np.uint32) * 256, np.arange(128, dtype=np.uint32) * 16],
        axis=1,
    )
    in_maps = []
    for core in range(8):
        b, h = divmod(core, 2)
        sl = slice(h * EH, (h + 1) * EH)
        in_maps.append({
            "x_in": x[b],
            "wmT_in": np.ascontiguousarray(W_mask[sl].T),
            "wup_in": W_up[sl],
            "bm_in": np.ascontiguousarray(b_mask[sl].reshape(G, 128).T),
            "pb_in": pb,
        })

    results = _get_runner()(in_maps)

    out = np.zeros((B, E, T), dtype=np.float32)
    for b in range(B):
        Z = 0.0
        for h in range(2):
            Z += np.asarray(results[2 * b + h]["s_out"], np.float64).sum()
        cand_val = []
        cand_pos = []
        cand_out = []
        for h in range(2):
            r = results[2 * b + h]
            ids = r["ids_out"].reshape(-1).astype(np.int64)        # [512]
            rww = r["raw_out"].reshape(128, 4, 128)
            expw = r["expw_out"].reshape(128, 4, 128).astype(np.float64)
            upw = r["upw_out"].reshape(128, 4, 128)
            g = (ids >> 4) & 15
            p = ids >> 8
            j = ids & 15
            e = h * EH + g * 128 + p                               # [512]
            t0 = j * 128
            upb = upw.reshape(512, 128) + b_up[e][:, None]
            vals = (expw.reshape(512, 128) / Z) * upb              # output values
            pos = (e[:, None] * T + (t0[:, None] + np.arange(128))).reshape(-1)
            cand_val.append(rww.reshape(-1))
            cand_pos.append(pos)
            cand_out.append(vals.reshape(-1))
        cv = np.concatenate(cand_val)
        cp = np.concatenate(cand_pos)
        co = np.concatenate(cand_out)
        # rank by exact fp32 logit, ties broken by lower flat index (jax order)
        order = np.lexsort((cp, -cv))[:K_SPARSE]
        out[b].reshape(-1)[cp[order]] = co[order].astype(np.float32)
    return out
